# revision 36
# baseline (speedup 1.0000x reference)
"""Distributed flash-attention kernel for Trainium2 (8 NeuronCores).

Problem: out = (softmax((x@wq.T)(x@wk.T)^T / sqrt(D)) @ (x@wv.T)) @ wo.T
with B=2, S=2048, C=2048, H=16 heads, D=128.

Sharding: head-parallel. Core i computes heads {2i, 2i+1} for both batches
(projections from per-head weight slices, full attention for its heads),
then an 8-way AllToAll redistributes attention outputs from head-sharded
to row-sharded layout, and each core runs the output projection for its
512 rows of the flattened [4096, 2048] output. The AllToAll is split into
two collectives (one per local head) so the first overlaps the second
head's attention compute and the second overlaps the first half of the
output projection.

Layouts (chosen so every matmul operand is in its natural on-chip
orientation, i.e. the contraction dim rides the SBUF partition axis):
  - x is fed pre-transposed per batch: xT[b] = x[b].T            [C, S]
  - weight shards are fed pre-transposed: wqT_i = wq[rows_i].T   [C, 256]
  - woT = wo.T                                                   [C, C]
  - attention scores are computed transposed: S^T[kpos, q]
  - attention output comes out transposed: O^T[d, q] which is exactly the
    channel-major layout the output projection wants for its lhsT.
"""

import numpy as np

import concourse.tile as tile
from concourse import bacc, mybir
from concourse.bass_utils import run_bass_kernel_spmd

B, S, C = 2, 2048, 2048
H, D = 16, 128
W = 8                      # cores
HPC = H // W               # heads per core (2)
HD = HPC * D               # per-core head channels (256)
SCALE = 1.0 / float(np.sqrt(D))

P = 128                    # SBUF partitions
NK = C // P                # 16 contraction tiles
NQ = S // 512              # 4 q-chunks of 512
NS = S // P                # 16 seq tiles of 128
ROWS = B * S // W          # 512 output rows per core

F32 = mybir.dt.float32

# matmul compute dtype. bfloat16 runs the PE with fast weight loads and
# halves DMA + collective traffic; accumulation stays fp32 in PSUM.
MMD = mybir.dt.bfloat16
MMD_NP = "bfloat16"


def build_nc():
    nc = bacc.Bacc("TRN2", target_bir_lowering=False, debug=False, num_devices=W)

    xT = nc.declare_dram_parameter("xT", [B, C, S], MMD, isOutput=False)
    wqT = nc.declare_dram_parameter("wqT", [C, HD], MMD, isOutput=False)
    wkT = nc.declare_dram_parameter("wkT", [C, HD], MMD, isOutput=False)
    wvT = nc.declare_dram_parameter("wvT", [C, HD], MMD, isOutput=False)
    woT = nc.declare_dram_parameter("woT", [C, C], MMD, isOutput=False)
    out = nc.declare_dram_parameter("out", [ROWS, C], F32, isOutput=True)

    with tile.TileContext(nc) as tc:
        with (
            tc.tile_pool(name="consts", bufs=1) as consts,
            tc.tile_pool(name="wpool", bufs=1) as wpool,
            tc.tile_pool(name="xpool", bufs=5) as xpool,
            tc.tile_pool(name="qkv", bufs=1) as qkv,
            tc.tile_pool(name="ptp", bufs=9) as ptp,
            tc.tile_pool(name="small", bufs=2) as small,
            tc.tile_pool(name="att", bufs=1) as attp,
            tc.tile_pool(name="wop", bufs=6) as wop,
            tc.tile_pool(name="outp", bufs=2) as outp,
            tc.tile_pool(name="psA", bufs=4, space="PSUM") as psA,
            tc.tile_pool(name="psB", bufs=2, space="PSUM") as psB,
            tc.tile_pool(name="dram", bufs=1, space="DRAM") as dram,
        ):
            ones_col = consts.tile([P, 1], MMD, name="ones_col")
            nc.vector.memset(ones_col[:], 1.0)

            # per-head A2A buffers: shard = [128 chan, 512 q]
            a2a_in = [dram.tile([W, D, 512], MMD, name=f"a2a_in{h}")
                      for h in range(HPC)]
            a2a_out = [dram.tile([W, D, 512], MMD, name=f"a2a_out{h}")
                       for h in range(HPC)]

            # ---- resident weight shards, DMA'd lazily inside the first
            # chunk's ct loop so the first matmuls start immediately.
            wq_sb, wk_sb, wv_sb = [], [], []
            for ct in range(NK):
                for lst, nm in ((wq_sb, "wq"), (wk_sb, "wk"), (wv_sb, "wv")):
                    t = wpool.tile([P, HD], MMD, tag=f"{nm}{ct}", name=f"{nm}{ct}")
                    lst.append(t)

            for b in range(B):
                # ---- phase 1: projections for batch b ----------------------
                # qt/kt: [d=128, S] per local head; v: [128, NS*HD] s-tile-major
                qt = [qkv.tile([P, S], MMD, tag=f"qt{h}", name=f"qt{h}") for h in range(HPC)]
                kt = [qkv.tile([P, S], MMD, tag=f"kt{h}", name=f"kt{h}") for h in range(HPC)]
                v_sb = qkv.tile([P, NS * HD], MMD, tag="v", name="v_sb")

                for sc in range(NQ):  # s-chunks of 512
                    q_ps = [psA.tile([P, 512], F32, tag="ps", name=f"qps{h}")
                            for h in range(HPC)]
                    k_ps = [psA.tile([P, 512], F32, tag="ps", name=f"kps{h}")
                            for h in range(HPC)]
                    v_ps = psB.tile([P, 1024], F32, tag="ps2", name="v_ps")
                    for ct in range(NK):
                        if b == 0 and sc == 0:
                            for wsb, wdr in ((wq_sb, wqT), (wk_sb, wkT),
                                             (wv_sb, wvT)):
                                nc.sync.dma_start(
                                    wsb[ct][:], wdr[ct * P:(ct + 1) * P, :])
                        xt = xpool.tile([P, 512], MMD, tag="xt")
                        nc.sync.dma_start(
                            xt[:],
                            xT[b, ct * P:(ct + 1) * P, sc * 512:(sc + 1) * 512],
                        )
                        st, sp = (ct == 0), (ct == NK - 1)
                        for h in range(HPC):
                            wsl = slice(h * D, (h + 1) * D)
                            nc.tensor.matmul(
                                q_ps[h][:], wq_sb[ct][:, wsl], xt[:],
                                start=st, stop=sp,
                            )
                            nc.tensor.matmul(
                                k_ps[h][:], wk_sb[ct][:, wsl], xt[:],
                                start=st, stop=sp,
                            )
                        for sti in range(4):  # s-tiles within chunk
                            # two [128,256] groups share each PSUM bank;
                            # start=True clears the whole bank, so only the
                            # first group per bank may assert it.
                            nc.tensor.matmul(
                                v_ps[:, sti * HD:(sti + 1) * HD],
                                xt[:, sti * P:(sti + 1) * P],
                                wv_sb[ct][:],
                                start=(st and sti % 2 == 0), stop=sp,
                            )
                    ssl = slice(sc * 512, (sc + 1) * 512)
                    nc.scalar.copy(qt[0][:, ssl], q_ps[0][:])
                    nc.vector.tensor_copy(qt[1][:, ssl], q_ps[1][:])
                    nc.scalar.copy(kt[0][:, ssl], k_ps[0][:])
                    nc.vector.tensor_copy(kt[1][:, ssl], k_ps[1][:])
                    nc.scalar.copy(
                        v_sb[:, sc * 1024:(sc + 1) * 1024], v_ps[:]
                    )

                # ---- phase 2: attention for each local head ----------------
                for h in range(HPC):
                    pending = None  # (h, den_ps, acc, o_ps, rb) of previous chunk

                    def flush_pending():
                        nonlocal pending
                        if pending is None:
                            return
                        p_hh, p_den, p_acc, p_ops, p_rb = pending
                        nc.tensor.matmul(p_den, ones_col[:], p_acc[:],
                                         start=True, stop=True)
                        o_sb = small.tile([P, 512], MMD, tag="osb", name="o_sb")
                        nc.vector.tensor_copy(o_sb[:], p_ops[:])
                        recip = small.tile([1, 512], F32, tag="recip", name="recip")
                        nc.vector.reciprocal_approx_fast(out=recip[:], in_=p_den)
                        bcast = small.tile([P, 512], F32, tag="bcast", name="bcast")
                        nc.gpsimd.partition_broadcast(bcast[:], recip[:])
                        nc.vector.tensor_mul(o_sb[:], o_sb[:], bcast[:])
                        nc.sync.dma_start(a2a_in[p_hh][p_rb, :, :], o_sb[:])
                        pending = None

                    for qc in range(NQ):
                        qsl = slice(qc * 512, (qc + 1) * 512)
                        den_full = psA.tile([P, 512], F32, tag="ps", name="den_full")
                        den_ps = den_full[0:1, :]
                        o_ps = psA.tile([P, 512], F32, tag="ps", name="o_ps")
                        pt = []

                        def av_mm(ki):
                            psrc = pt[ki // 2][:, (ki % 2) * 512:(ki % 2 + 1) * 512]
                            nc.tensor.matmul(
                                o_ps[:],
                                v_sb[:, ki * HD + h * D: ki * HD + (h + 1) * D],
                                psrc,
                                start=(ki == 0), stop=(ki == NS - 1),
                            )

                        for kp in range(NS // 2):  # pairs of kpos-tiles
                            s_ps = psB.tile([P, 1024], F32, tag="ps2", name="s_ps")
                            for j in range(2):
                                ki = kp * 2 + j
                                nc.tensor.matmul(
                                    s_ps[:, j * 512:(j + 1) * 512],
                                    kt[h][:, ki * P:(ki + 1) * P],
                                    qt[h][:, qsl],
                                    start=True, stop=True,
                                )
                            pe = ptp.tile([P, 1024], MMD, tag="pt")
                            nc.scalar.activation(
                                pe[:], s_ps[:],
                                mybir.ActivationFunctionType.Exp,
                                scale=SCALE,
                            )
                            pt.append(pe)
                            if kp == 1:
                                # previous chunk's epilogue, hidden under the
                                # early score matmuls
                                flush_pending()
                            # AV of the pair exp'd two steps ago keeps the PE
                            # busy while ACT works through the exp backlog
                            if kp >= 2:
                                av_mm(2 * (kp - 2))
                                av_mm(2 * (kp - 2) + 1)
                        # denominator partial sums on DVE (keeps PE free):
                        # acc[q] accumulates all 16 kpos-tiles, then one tiny
                        # PE matmul folds the 128 partitions.
                        # ping-pong accumulators (in-place blocks DVE 2x mode)
                        aps = [small.tile([P, 512], MMD, tag=f"acc{j}", name=f"acc{j}")
                               for j in range(2)]
                        nc.vector.tensor_add(aps[0][:], pt[0][:, 0:512], pt[0][:, 512:1024])
                        n_add = 1
                        for kp in range(1, NS // 2):
                            for half in range(2):
                                src_ap = pt[kp][:, half * 512:(half + 1) * 512]
                                nc.vector.tensor_add(
                                    aps[n_add % 2][:], aps[(n_add + 1) % 2][:], src_ap)
                                n_add += 1
                        acc = aps[(n_add + 1) % 2]
                        for ki in range(2 * (NS // 2 - 2), NS):
                            av_mm(ki)
                        rb = b * NQ + qc  # destination core / row-block
                        pending = (h, den_ps, acc, o_ps, rb)
                    flush_pending()

                    # head-h exchange fires once both batches' data for this
                    # head is in; h0's collective overlaps h1's attention.
                    if b == B - 1:
                        nc.gpsimd.collective_compute(
                            "AllToAll",
                            mybir.AluOpType.bypass,
                            replica_groups=[list(range(W))],
                            ins=[a2a_in[h].opt()],
                            outs=[a2a_out[h].opt()],
                        )

            # ---- phase 3: output projection --------------------------------
            attT = {}
            for par in range(HPC):        # even c-tiles (head0) first
                for g in range(W):
                    ct = 2 * g + par
                    at = attp.tile([P, 512], MMD, tag=f"attT{ct}", name=f"attT{ct}")
                    nc.sync.dma_start(at[:], a2a_out[par][g, :, :])
                    attT[ct] = at

            # Even c-tiles (exchanged by the first collective) accumulate into
            # SBUF for all four output chunks while the second collective is
            # still in flight; odd c-tiles then accumulate in PSUM and the
            # two halves are summed on the way out.
            acc_tags = ["qt0", "qt1", "kt0", "kt1"]  # dead after attention
            acc = []
            for oc in range(4):
                a = qkv.tile([P, 4 * 512], F32, tag=acc_tags[oc], name=f"acc{oc}")
                acc.append(a)
            for oc in range(4):
                pair = [psB.tile([P, 1024], F32, tag="ps2", name=f"opse_{oc}_{j}") for j in range(2)]
                o_ps2 = [pair[j // 2][:, (j % 2) * 512:(j % 2 + 1) * 512] for j in range(4)]
                for idx, g in enumerate(range(W)):
                    ct = 2 * g
                    wo_t = wop.tile([P, 512], MMD, tag="wo")
                    nc.sync.dma_start(
                        wo_t[:],
                        woT[ct * P:(ct + 1) * P, oc * 512:(oc + 1) * 512],
                    )
                    for qi in range(4):
                        nc.tensor.matmul(
                            o_ps2[qi][:],
                            attT[ct][:, qi * P:(qi + 1) * P],
                            wo_t[:],
                            start=(idx == 0), stop=(idx == W - 1),
                        )
                for qi in range(4):
                    nc.vector.tensor_copy(
                        acc[oc][:, qi * 512:(qi + 1) * 512], o_ps2[qi][:],
                    )
            for oc in range(4):
                pair = [psB.tile([P, 1024], F32, tag="ps2", name=f"opso_{oc}_{j}") for j in range(2)]
                o_ps2 = [pair[j // 2][:, (j % 2) * 512:(j % 2 + 1) * 512] for j in range(4)]
                for idx, g in enumerate(range(W)):
                    ct = 2 * g + 1
                    wo_t = wop.tile([P, 512], MMD, tag="wo")
                    nc.sync.dma_start(
                        wo_t[:],
                        woT[ct * P:(ct + 1) * P, oc * 512:(oc + 1) * 512],
                    )
                    for qi in range(4):
                        nc.tensor.matmul(
                            o_ps2[qi][:],
                            attT[ct][:, qi * P:(qi + 1) * P],
                            wo_t[:],
                            start=(idx == 0), stop=(idx == W - 1),
                        )
                for qi in range(4):
                    ot = outp.tile([P, 512], F32, tag="ot")
                    nc.vector.tensor_add(
                        ot[:], o_ps2[qi][:],
                        acc[oc][:, qi * 512:(qi + 1) * 512],
                    )
                    nc.sync.dma_start(
                        out[qi * P:(qi + 1) * P, oc * 512:(oc + 1) * 512], ot[:]
                    )

    nc.compile()
    return nc


_NC_CACHE = None


def _get_nc():
    global _NC_CACHE
    if _NC_CACHE is None:
        _NC_CACHE = build_nc()
    return _NC_CACHE


def make_in_maps(x, wq, wk, wv, wo):
    import ml_dtypes
    bf16 = ml_dtypes.bfloat16
    xT = np.ascontiguousarray(np.transpose(np.asarray(x, np.float32), (0, 2, 1))).astype(bf16)
    woT = np.ascontiguousarray(np.asarray(wo, np.float32).T).astype(bf16)
    in_maps = []
    for i in range(W):
        rows = slice(i * HD, (i + 1) * HD)
        in_maps.append({
            "xT": xT,
            "wqT": np.ascontiguousarray(np.asarray(wq, np.float32)[rows].T).astype(bf16),
            "wkT": np.ascontiguousarray(np.asarray(wk, np.float32)[rows].T).astype(bf16),
            "wvT": np.ascontiguousarray(np.asarray(wv, np.float32)[rows].T).astype(bf16),
            "woT": woT,
        })
    return in_maps


def kernel(x, wq, wk, wv, wo, _trace=False):
    nc = _get_nc()
    in_maps = make_in_maps(x, wq, wk, wv, wo)
    last_err = None
    for _attempt in range(3):
        try:
            res = run_bass_kernel_spmd(
                nc, in_maps, core_ids=list(range(W)), trace=_trace)
            break
        except Exception as e:  # transient NRT device errors: retry
            last_err = e
    else:
        raise last_err
    flat = np.concatenate([res.results[i]["out"] for i in range(W)], axis=0)
    out = flat.reshape(B, S, C)
    if _trace:
        kernel.last_exec_time_ns = res.exec_time_ns
        kernel.last_profile = res
    return out


if __name__ == "__main__":
    rng = np.random.default_rng(0)
    x = rng.standard_normal((B, S, C), dtype=np.float32)
    wq, wk, wv, wo = (rng.standard_normal((C, C), dtype=np.float32) / np.sqrt(C)
                      for _ in range(4))
    got = kernel(x, wq, wk, wv, wo)
    print("out", got.shape, got.dtype)


# revision 37
# speedup vs baseline: 1.0394x; 1.0394x over previous
"""Distributed flash-attention kernel for Trainium2 (8 NeuronCores).

Problem: out = (softmax((x@wq.T)(x@wk.T)^T / sqrt(D)) @ (x@wv.T)) @ wo.T
with B=2, S=2048, C=2048, H=16 heads, D=128.

Sharding: head-parallel. Core i computes heads {2i, 2i+1} for both batches
(projections from per-head weight slices, full attention for its heads),
then an 8-way AllToAll redistributes attention outputs from head-sharded
to row-sharded layout, and each core runs the output projection for its
512 rows of the flattened [4096, 2048] output. The AllToAll is split into
two collectives (one per local head) so the first overlaps the second
head's attention compute and the second overlaps the first half of the
output projection.

Layouts (chosen so every matmul operand is in its natural on-chip
orientation, i.e. the contraction dim rides the SBUF partition axis):
  - x is fed pre-transposed per batch: xT[b] = x[b].T            [C, S]
  - weight shards are fed pre-transposed: wqT_i = wq[rows_i].T   [C, 256]
  - woT = wo.T                                                   [C, C]
  - attention scores are computed transposed: S^T[kpos, q]
  - attention output comes out transposed: O^T[d, q] which is exactly the
    channel-major layout the output projection wants for its lhsT.
"""

import numpy as np

import concourse.tile as tile
from concourse import bacc, mybir
from concourse.bass_utils import run_bass_kernel_spmd

B, S, C = 2, 2048, 2048
H, D = 16, 128
W = 8                      # cores
HPC = H // W               # heads per core (2)
HD = HPC * D               # per-core head channels (256)
SCALE = 1.0 / float(np.sqrt(D))

P = 128                    # SBUF partitions
NK = C // P                # 16 contraction tiles
NQ = S // 512              # 4 q-chunks of 512
NS = S // P                # 16 seq tiles of 128
ROWS = B * S // W          # 512 output rows per core

F32 = mybir.dt.float32

# matmul compute dtype. bfloat16 runs the PE with fast weight loads and
# halves DMA + collective traffic; accumulation stays fp32 in PSUM.
MMD = mybir.dt.bfloat16
MMD_NP = "bfloat16"


def build_nc():
    nc = bacc.Bacc("TRN2", target_bir_lowering=False, debug=False, num_devices=W)

    xT = nc.declare_dram_parameter("xT", [B, C, S], MMD, isOutput=False)
    wqT = nc.declare_dram_parameter("wqT", [C, HD], MMD, isOutput=False)
    wkT = nc.declare_dram_parameter("wkT", [C, HD], MMD, isOutput=False)
    wvT = nc.declare_dram_parameter("wvT", [C, HD], MMD, isOutput=False)
    woT = nc.declare_dram_parameter("woT", [C, C], MMD, isOutput=False)
    out = nc.declare_dram_parameter("out", [ROWS, C], F32, isOutput=True)

    with tile.TileContext(nc) as tc:
        with (
            tc.tile_pool(name="consts", bufs=1) as consts,
            tc.tile_pool(name="wpool", bufs=1) as wpool,
            tc.tile_pool(name="xpool", bufs=5) as xpool,
            tc.tile_pool(name="qkv", bufs=1) as qkv,
            tc.tile_pool(name="ptp", bufs=10) as ptp,
            tc.tile_pool(name="small", bufs=3) as small,
            tc.tile_pool(name="att", bufs=1) as attp,
            tc.tile_pool(name="wop", bufs=6) as wop,
            tc.tile_pool(name="outp", bufs=2) as outp,
            tc.tile_pool(name="psA", bufs=4, space="PSUM") as psA,
            tc.tile_pool(name="psB", bufs=2, space="PSUM") as psB,
            tc.tile_pool(name="dram", bufs=1, space="DRAM") as dram,
        ):
            ones_col = consts.tile([P, 1], MMD, name="ones_col")
            nc.vector.memset(ones_col[:], 1.0)

            # per-head A2A buffers: shard = [128 chan, 512 q]
            a2a_in = [dram.tile([W, D, 512], MMD, name=f"a2a_in{h}")
                      for h in range(HPC)]
            a2a_out = [dram.tile([W, D, 512], MMD, name=f"a2a_out{h}")
                       for h in range(HPC)]

            # ---- resident weight shards, DMA'd lazily inside the first
            # chunk's ct loop so the first matmuls start immediately.
            wq_sb, wk_sb, wv_sb = [], [], []
            for ct in range(NK):
                for lst, nm in ((wq_sb, "wq"), (wk_sb, "wk"), (wv_sb, "wv")):
                    t = wpool.tile([P, HD], MMD, tag=f"{nm}{ct}", name=f"{nm}{ct}")
                    lst.append(t)

            for b in range(B):
                # ---- phase 1: projections for batch b ----------------------
                # qt/kt: [d=128, S] per local head; v: [128, NS*HD] s-tile-major
                qt = [qkv.tile([P, S], MMD, tag=f"qt{h}", name=f"qt{h}") for h in range(HPC)]
                kt = [qkv.tile([P, S], MMD, tag=f"kt{h}", name=f"kt{h}") for h in range(HPC)]
                v_sb = qkv.tile([P, NS * HD], MMD, tag="v", name="v_sb")

                for sc in range(NQ):  # s-chunks of 512
                    q_ps = [psA.tile([P, 512], F32, tag="ps", name=f"qps{h}")
                            for h in range(HPC)]
                    k_ps = [psA.tile([P, 512], F32, tag="ps", name=f"kps{h}")
                            for h in range(HPC)]
                    v_ps = psB.tile([P, 1024], F32, tag="ps2", name="v_ps")
                    for ct in range(NK):
                        if b == 0 and sc == 0:
                            for wsb, wdr in ((wq_sb, wqT), (wk_sb, wkT),
                                             (wv_sb, wvT)):
                                nc.sync.dma_start(
                                    wsb[ct][:], wdr[ct * P:(ct + 1) * P, :])
                        xt = xpool.tile([P, 512], MMD, tag="xt")
                        nc.sync.dma_start(
                            xt[:],
                            xT[b, ct * P:(ct + 1) * P, sc * 512:(sc + 1) * 512],
                        )
                        st, sp = (ct == 0), (ct == NK - 1)
                        for h in range(HPC):
                            wsl = slice(h * D, (h + 1) * D)
                            nc.tensor.matmul(
                                q_ps[h][:], wq_sb[ct][:, wsl], xt[:],
                                start=st, stop=sp,
                            )
                            nc.tensor.matmul(
                                k_ps[h][:], wk_sb[ct][:, wsl], xt[:],
                                start=st, stop=sp,
                            )
                        for sti in range(4):  # s-tiles within chunk
                            # two [128,256] groups share each PSUM bank;
                            # start=True clears the whole bank, so only the
                            # first group per bank may assert it.
                            nc.tensor.matmul(
                                v_ps[:, sti * HD:(sti + 1) * HD],
                                xt[:, sti * P:(sti + 1) * P],
                                wv_sb[ct][:],
                                start=(st and sti % 2 == 0), stop=sp,
                            )
                    ssl = slice(sc * 512, (sc + 1) * 512)
                    nc.scalar.copy(qt[0][:, ssl], q_ps[0][:])
                    nc.vector.tensor_copy(qt[1][:, ssl], q_ps[1][:])
                    nc.scalar.copy(kt[0][:, ssl], k_ps[0][:])
                    nc.vector.tensor_copy(kt[1][:, ssl], k_ps[1][:])
                    nc.scalar.copy(
                        v_sb[:, sc * 1024:(sc + 1) * 1024], v_ps[:]
                    )

                # ---- phase 2: attention for each local head ----------------
                for h in range(HPC):
                    pending = None  # (h, den_ps, acc, o_ps, rb) of previous chunk

                    def flush_pending():
                        nonlocal pending
                        if pending is None:
                            return
                        p_hh, p_den, p_acc, p_ops, p_rb = pending
                        nc.tensor.matmul(p_den, ones_col[:], p_acc[:],
                                         start=True, stop=True)
                        o_sb = small.tile([P, 512], MMD, tag="osb", name="o_sb")
                        nc.vector.tensor_copy(o_sb[:], p_ops[:])
                        recip = small.tile([1, 512], F32, tag="recip", name="recip")
                        nc.vector.reciprocal_approx_fast(out=recip[:], in_=p_den)
                        bcast = small.tile([P, 512], F32, tag="bcast", name="bcast")
                        nc.gpsimd.partition_broadcast(bcast[:], recip[:])
                        nc.vector.tensor_mul(o_sb[:], o_sb[:], bcast[:])
                        nc.sync.dma_start(a2a_in[p_hh][p_rb, :, :], o_sb[:])
                        pending = None

                    for qc in range(NQ):
                        qsl = slice(qc * 512, (qc + 1) * 512)
                        den_full = psA.tile([P, 512], F32, tag="ps", name="den_full")
                        den_ps = den_full[0:1, :]
                        o_ps = psA.tile([P, 512], F32, tag="ps", name="o_ps")
                        pt = []

                        def av_mm(ki):
                            psrc = pt[ki // 2][:, (ki % 2) * 512:(ki % 2 + 1) * 512]
                            nc.tensor.matmul(
                                o_ps[:],
                                v_sb[:, ki * HD + h * D: ki * HD + (h + 1) * D],
                                psrc,
                                start=(ki == 0), stop=(ki == NS - 1),
                            )

                        for kp in range(NS // 2):  # pairs of kpos-tiles
                            s_ps = psB.tile([P, 1024], F32, tag="ps2", name="s_ps")
                            for j in range(2):
                                ki = kp * 2 + j
                                nc.tensor.matmul(
                                    s_ps[:, j * 512:(j + 1) * 512],
                                    kt[h][:, ki * P:(ki + 1) * P],
                                    qt[h][:, qsl],
                                    start=True, stop=True,
                                )
                            pe = ptp.tile([P, 1024], MMD, tag="pt")
                            nc.scalar.activation(
                                pe[:], s_ps[:],
                                mybir.ActivationFunctionType.Exp,
                                scale=SCALE,
                            )
                            pt.append(pe)
                            if kp == 1:
                                # previous chunk's epilogue, hidden under the
                                # early score matmuls
                                flush_pending()
                            # AV of the pair exp'd two steps ago keeps the PE
                            # busy while ACT works through the exp backlog
                            if kp >= 2:
                                av_mm(2 * (kp - 2))
                                av_mm(2 * (kp - 2) + 1)
                        # denominator partial sums on DVE (keeps PE free):
                        # acc[q] accumulates all 16 kpos-tiles, then one tiny
                        # PE matmul folds the 128 partitions.
                        # ping-pong accumulators (in-place blocks DVE 2x mode)
                        aps = [small.tile([P, 512], MMD, tag=f"acc{j}", name=f"acc{j}")
                               for j in range(2)]
                        nc.vector.tensor_add(aps[0][:], pt[0][:, 0:512], pt[0][:, 512:1024])
                        n_add = 1
                        for kp in range(1, NS // 2):
                            for half in range(2):
                                src_ap = pt[kp][:, half * 512:(half + 1) * 512]
                                nc.vector.tensor_add(
                                    aps[n_add % 2][:], aps[(n_add + 1) % 2][:], src_ap)
                                n_add += 1
                        acc = aps[(n_add + 1) % 2]
                        for ki in range(2 * (NS // 2 - 2), NS):
                            av_mm(ki)
                        rb = b * NQ + qc  # destination core / row-block
                        pending = (h, den_ps, acc, o_ps, rb)
                    flush_pending()

                    # head-h exchange fires once both batches' data for this
                    # head is in; h0's collective overlaps h1's attention.
                    if b == B - 1:
                        nc.gpsimd.collective_compute(
                            "AllToAll",
                            mybir.AluOpType.bypass,
                            replica_groups=[list(range(W))],
                            ins=[a2a_in[h].opt()],
                            outs=[a2a_out[h].opt()],
                        )

            # ---- phase 3: output projection --------------------------------
            attT = {}
            for par in range(HPC):        # even c-tiles (head0) first
                for g in range(W):
                    ct = 2 * g + par
                    at = attp.tile([P, 512], MMD, tag=f"attT{ct}", name=f"attT{ct}")
                    nc.sync.dma_start(at[:], a2a_out[par][g, :, :])
                    attT[ct] = at

            # Even c-tiles (exchanged by the first collective) accumulate into
            # SBUF for all four output chunks while the second collective is
            # still in flight; odd c-tiles then accumulate in PSUM and the
            # two halves are summed on the way out.
            acc_tags = ["qt0", "qt1", "kt0", "kt1"]  # dead after attention
            acc = []
            for oc in range(4):
                a = qkv.tile([P, 4 * 512], F32, tag=acc_tags[oc], name=f"acc{oc}")
                acc.append(a)
            for oc in range(4):
                pair = [psB.tile([P, 1024], F32, tag="ps2", name=f"opse_{oc}_{j}") for j in range(2)]
                o_ps2 = [pair[j // 2][:, (j % 2) * 512:(j % 2 + 1) * 512] for j in range(4)]
                for idx, g in enumerate(range(W)):
                    ct = 2 * g
                    wo_t = wop.tile([P, 512], MMD, tag="wo")
                    nc.sync.dma_start(
                        wo_t[:],
                        woT[ct * P:(ct + 1) * P, oc * 512:(oc + 1) * 512],
                    )
                    for qi in range(4):
                        nc.tensor.matmul(
                            o_ps2[qi][:],
                            attT[ct][:, qi * P:(qi + 1) * P],
                            wo_t[:],
                            start=(idx == 0), stop=(idx == W - 1),
                        )
                for qi in range(4):
                    nc.vector.tensor_copy(
                        acc[oc][:, qi * 512:(qi + 1) * 512], o_ps2[qi][:],
                    )
            for oc in range(4):
                pair = [psB.tile([P, 1024], F32, tag="ps2", name=f"opso_{oc}_{j}") for j in range(2)]
                o_ps2 = [pair[j // 2][:, (j % 2) * 512:(j % 2 + 1) * 512] for j in range(4)]
                for idx, g in enumerate(range(W)):
                    ct = 2 * g + 1
                    wo_t = wop.tile([P, 512], MMD, tag="wo")
                    nc.sync.dma_start(
                        wo_t[:],
                        woT[ct * P:(ct + 1) * P, oc * 512:(oc + 1) * 512],
                    )
                    for qi in range(4):
                        nc.tensor.matmul(
                            o_ps2[qi][:],
                            attT[ct][:, qi * P:(qi + 1) * P],
                            wo_t[:],
                            start=(idx == 0), stop=(idx == W - 1),
                        )
                for qi in range(4):
                    ot = outp.tile([P, 512], F32, tag="ot")
                    nc.vector.tensor_add(
                        ot[:], o_ps2[qi][:],
                        acc[oc][:, qi * 512:(qi + 1) * 512],
                    )
                    nc.sync.dma_start(
                        out[qi * P:(qi + 1) * P, oc * 512:(oc + 1) * 512], ot[:]
                    )

    nc.compile()
    return nc


_NC_CACHE = None


def _get_nc():
    global _NC_CACHE
    if _NC_CACHE is None:
        _NC_CACHE = build_nc()
    return _NC_CACHE


def make_in_maps(x, wq, wk, wv, wo):
    import ml_dtypes
    bf16 = ml_dtypes.bfloat16
    xT = np.ascontiguousarray(np.transpose(np.asarray(x, np.float32), (0, 2, 1))).astype(bf16)
    woT = np.ascontiguousarray(np.asarray(wo, np.float32).T).astype(bf16)
    in_maps = []
    for i in range(W):
        rows = slice(i * HD, (i + 1) * HD)
        in_maps.append({
            "xT": xT,
            "wqT": np.ascontiguousarray(np.asarray(wq, np.float32)[rows].T).astype(bf16),
            "wkT": np.ascontiguousarray(np.asarray(wk, np.float32)[rows].T).astype(bf16),
            "wvT": np.ascontiguousarray(np.asarray(wv, np.float32)[rows].T).astype(bf16),
            "woT": woT,
        })
    return in_maps


def kernel(x, wq, wk, wv, wo, _trace=False):
    nc = _get_nc()
    in_maps = make_in_maps(x, wq, wk, wv, wo)
    last_err = None
    for _attempt in range(3):
        try:
            res = run_bass_kernel_spmd(
                nc, in_maps, core_ids=list(range(W)), trace=_trace)
            break
        except Exception as e:  # transient NRT device errors: retry
            last_err = e
    else:
        raise last_err
    flat = np.concatenate([res.results[i]["out"] for i in range(W)], axis=0)
    out = flat.reshape(B, S, C)
    if _trace:
        kernel.last_exec_time_ns = res.exec_time_ns
        kernel.last_profile = res
    return out


if __name__ == "__main__":
    rng = np.random.default_rng(0)
    x = rng.standard_normal((B, S, C), dtype=np.float32)
    wq, wk, wv, wo = (rng.standard_normal((C, C), dtype=np.float32) / np.sqrt(C)
                      for _ in range(4))
    got = kernel(x, wq, wk, wv, wo)
    print("out", got.shape, got.dtype)
